# revision 10
# baseline (speedup 1.0000x reference)
"""Bahdanau additive attention on 8 Trainium2 NeuronCores.

c[b] = softmax_t( tanh(s@W_a + h@U_a) @ v_a ) @ h[b]

Sharding: data-parallel over batch B=32 -> 4 batches per core; W_a, U_a,
v_a replicated. Unnormalized softmax (scores bounded by ||v_a||_1, so
f32 exp never overflows; no running max needed).

Layout: t = i*1024 + r*128 + tq (tq minor, on partitions). Per half-chunk
(512 t): 4 full-row cast DMAs f32->bf16, 4 XBAR transposes of the bf16
dh[0:768] part, and (f8 mode) a DVE cast of dh[768:1024] to fp8e4 plus a
16-bit-pair XBAR transpose whose dh-pair interleave lands exactly in the
[K,2,N] layout DoubleRow matmuls want (k = 768 + 2p + i).

Per chunk: mm1 = per (at, 256-col region): fp8 DR matmul (start) + 6 bf16
o-tile matmuls (stop); tanh on ACT with W_a@s bias; e-dot matmuls
interleaved one a-tile behind mm1 so their weight loads hide under mm1
streams; exp -> pt bf16 with accum_out partial denominators; mm3
(c += pt.T @ hbf) deferred one chunk to cover load/transpose latency.

Setup avoids SWDGE descriptor storms: s/v transposed on the PE against
tiny identity matrices; only U (and the fp8 U quarter) uses cast DMAs.
"""

import numpy as np

B, T, DH, DS, DA = 32, 4096, 1024, 1024, 512
NCORES = 8
BL = B // NCORES          # batches per core
CHUNK_T = 1024            # timesteps per compute chunk
TH2 = 4                   # 128-row groups per half-chunk
P = 128

_CACHE = {}


def _legalize_waits(nc):
    """This walrus build allows at most one sync wait per instruction.
    Tile's tail drain (and any instruction whose operands arrive via two
    DMA lanes) can carry several; split the extras onto single-wait nops
    emitted just before, in the same engine's stream."""
    from concourse import mybir

    eng_map = {}
    for eng_name in ("sync", "tensor", "vector", "scalar", "gpsimd"):
        eng = getattr(nc, eng_name)
        eng_map[eng.engine] = eng

    def make_nop(engine_type):
        bi = eng_map[engine_type].nop(nofuse=True)
        inst = bi.ins
        for fn in nc.m.functions:
            for blk in fn.blocks:
                il = list(blk.instructions)
                if il and il[-1].name == inst.name:
                    blk.instructions = il[:-1]
                    return inst
        raise RuntimeError("nop not found after emit")

    for fn in nc.m.functions:
        for blk in fn.blocks:
            insts = list(blk.instructions)
            if not any(
                getattr(i, "sync_info", None) is not None
                and len(i.sync_info.on_wait) > 1
                for i in insts
            ):
                continue
            out = []
            for inst in insts:
                si = getattr(inst, "sync_info", None)
                if si is not None and len(si.on_wait) > 1:
                    waits = list(si.on_wait)
                    for w in waits[:-1]:
                        nop = make_nop(inst.engine)
                        nop.sync_info = mybir.SyncInfo(
                            on_wait=[w], on_update=[]
                        )
                        out.append(nop)
                    inst.sync_info = mybir.SyncInfo(
                        on_wait=[waits[-1]], on_update=list(si.on_update)
                    )
                out.append(inst)
            blk.instructions = out


def build_bass(bl=BL, t_total=T, f8=True):
    import concourse.bass as bass
    import concourse.tile as tile
    from concourse import mybir

    f32 = mybir.dt.float32
    bf16 = mybir.dt.bfloat16
    fp8 = mybir.dt.float8e4
    Alu = mybir.AluOpType
    Act = mybir.ActivationFunctionType
    Axis = mybir.AxisListType
    DR = mybir.MatmulPerfMode.DoubleRow

    nchunk = t_total // CHUNK_T
    DH8 = 256 if f8 else 0    # dh columns computed in fp8 DoubleRow
    DHB = DH - DH8            # dh columns computed in bf16
    OB = DHB // P             # bf16 o-tiles (6 or 8)
    NAT = DA // P             # a-tiles (4)

    nc = bass.Bass()
    s_ext = nc.declare_dram_parameter("s", [bl, DS], f32, isOutput=False)
    h_ext = nc.declare_dram_parameter("h", [bl, t_total, DH], f32, isOutput=False)
    w_ext = nc.declare_dram_parameter("W_a", [DS, DA], f32, isOutput=False)
    u_ext = nc.declare_dram_parameter("U_a", [DH, DA], f32, isOutput=False)
    v_ext = nc.declare_dram_parameter("v_a", [DA], f32, isOutput=False)
    out_ext = nc.declare_dram_parameter("out", [bl, DH], f32, isOutput=True)

    with tile.TileContext(nc) as tc:
        from contextlib import ExitStack

        with ExitStack() as ctx:
            singles = ctx.enter_context(tc.tile_pool(name="singles", bufs=1))
            hpool = ctx.enter_context(tc.tile_pool(name="hpool", bufs=6))
            htpool = ctx.enter_context(tc.tile_pool(name="htpool", bufs=6))
            if f8:
                h8pool = ctx.enter_context(tc.tile_pool(name="h8pool", bufs=6))
                ht8pool = ctx.enter_context(
                    tc.tile_pool(name="ht8pool", bufs=6)
                )
            tanhpool = ctx.enter_context(tc.tile_pool(name="tanhpool", bufs=5))
            smpool = ctx.enter_context(tc.tile_pool(name="smpool", bufs=4))
            outpool = ctx.enter_context(tc.tile_pool(name="outpool", bufs=2))
            mm1ps = ctx.enter_context(
                tc.tile_pool(name="mm1ps", bufs=2, space="PSUM")
            )
            cps_pool = ctx.enter_context(
                tc.tile_pool(name="cps", bufs=1, space="PSUM")
            )
            eps_pool = ctx.enter_context(
                tc.tile_pool(name="epsp", bufs=2, space="PSUM")
            )

            # ---- one-time setup (all small; emitted before h loads so
            # the first chunk's compute inputs are ready ASAP) ----
            # U bf16 part: [dh_lo, o, a]
            u_sb = singles.tile([P, OB, DA], bf16)
            u_re = u_ext[:].rearrange("(o p) a -> p o a", p=P)
            for o in range(OB):
                nc.gpsimd.dma_start(u_sb[:, o, :], u_re[:, o, :])
            if f8:
                # U fp8 quarter: u8[p, i, a] = U[DHB + 2p + i, a]
                u8_sb = singles.tile([P, 2, DA], fp8)
                u8_re = u_ext[DHB:DH, :].rearrange(
                    "(p two) a -> two p a", two=2
                )
                for i in range(2):
                    nc.gpsimd.dma_start(u8_sb[:, i, :], u8_re[i])

            w_sb = singles.tile([P, DS // P, DA], f32)
            nc.sync.dma_start(w_sb, w_ext[:].rearrange("(o p) a -> p o a", p=P))
            s_row = singles.tile([bl, DS], f32)
            nc.sync.dma_start(s_row, s_ext[:])
            v_row = singles.tile([1, DA], f32)
            nc.sync.dma_start(v_row, v_ext[:].rearrange("(x a) -> x a", x=1))

            ones_b = singles.tile([bl, bl], f32)
            nc.any.memset(ones_b, 1.0)
            identb = singles.tile([bl, bl], f32)
            nc.gpsimd.affine_select(
                identb,
                ones_b,
                pattern=[[-1, bl]],
                compare_op=Alu.is_equal,
                fill=0.0,
                base=0,
                channel_multiplier=1,
            )
            ident1 = singles.tile([1, 1], f32)
            nc.any.memset(ident1, 1.0)
            ones_sb = singles.tile([P, 1], f32)
            nc.any.memset(ones_sb, 1.0)

            # sT via PE transpose: st[ds_lo, o, b] = s[b, o*128+ds_lo]
            ps_st = eps_pool.tile([P, DS // P, bl], f32, name="ps_st", tag="eps")
            for o in range(DS // P):
                nc.tensor.transpose(
                    ps_st[:, o, :], s_row[:, o * P : (o + 1) * P], identb
                )
            st_sb = singles.tile([P, DS // P, bl], f32)
            nc.vector.tensor_copy(st_sb, ps_st)

            # v as [a_lo, a_hi] via PE transpose
            ps_v = eps_pool.tile([P, DA // P], f32, name="ps_v", tag="eps")
            for g in range(DA // P):
                nc.tensor.transpose(
                    ps_v[:, g : g + 1], v_row[:, g * P : (g + 1) * P], ident1
                )
            v_bf = singles.tile([P, DA // P], bf16)
            nc.vector.tensor_copy(v_bf, ps_v)

            # W_a_s^T [a_lo, at, b]
            ps_ws = eps_pool.tile([P, NAT, bl], f32, name="ps_ws", tag="eps")
            for at in range(NAT):
                for o in range(DS // P):
                    nc.tensor.matmul(
                        ps_ws[:, at, :],
                        w_sb[:, o, at * P : (at + 1) * P],
                        st_sb[:, o, :],
                        start=(o == 0),
                        stop=(o == DS // P - 1),
                    )
            ws_sb = singles.tile([P, NAT, bl], f32)
            nc.vector.tensor_copy(ws_sb, ps_ws)

            # ---- pipelined h loads (half-chunk = 512 t granularity) ----
            def emit_load(b, i, j):
                hbf = hpool.tile([P, TH2, DH], bf16, tag="hbf")
                t0 = i * CHUNK_T + j * TH2 * P
                for r in range(TH2):
                    nc.gpsimd.dma_start(
                        hbf[:, r, :],
                        h_ext[b, t0 + r * P : t0 + (r + 1) * P, :],
                    )
                # bf16 transposes, one per r (keeps the XBAR input 2D):
                # ht[dh_lo, r, o, tq] = h[t0 + r*128 + tq, o*128 + dh_lo]
                ht = htpool.tile([P, TH2, OB, P], bf16, tag="ht")
                for r in range(TH2):
                    nc.sync.dma_start_transpose(
                        ht[:, r, :, :], hbf[:, r, 0:DHB]
                    )
                if not f8:
                    return {"hbf": hbf, "ht": ht}
                # fp8 quarter: DVE cast, then 16-bit-pair XBAR transpose.
                h8 = h8pool.tile([P, TH2, DH8], fp8, tag="h8")
                nc.vector.tensor_copy(h8, hbf[:, :, DHB:DH])
                # ht8_16[p, r, tq] = pair(h8[tq, r, 2p], h8[tq, r, 2p+1])
                ht8 = ht8pool.tile([P, TH2, P], bf16, tag="ht8")
                nc.sync.dma_start_transpose(
                    ht8, h8.bitcast(bf16).rearrange("p a b -> p (a b)")
                )
                return {"hbf": hbf, "ht": ht, "ht8": ht8.bitcast(fp8)}

            halves = [
                (b, i, j)
                for b in range(bl)
                for i in range(nchunk)
                for j in range(2)
            ]
            loaded = {}
            next_load = 0

            def pump(upto):
                nonlocal next_load
                while next_load < min(upto, len(halves)):
                    key = halves[next_load]
                    loaded[key] = emit_load(*key)
                    next_load += 1

            pump(5)

            # ---- per-chunk compute ----
            def emit_mm1(ps1, tiles, at):
                # weight-reuse order: each lhsT (u8, then each o-tile) sweeps
                # all 4 output regions before the next weight load.
                aslice = slice(at * P, (at + 1) * P)
                if f8:
                    # one accumulation group per 512-col PSUM bank; the DR
                    # matmul for the first 256-half opens it (pending-zero
                    # covers the whole bank), everything else accumulates.
                    for j in range(2):
                        for q in range(2):
                            rhs8 = (
                                tiles[j]["ht8"][:, 2 * q : 2 * q + 2, :]
                                .rearrange("p r (n i) -> p i (r n)", i=2)
                            )
                            nc.tensor.matmul(
                                ps1[
                                    :,
                                    j * 512 + q * 256 : j * 512 + (q + 1) * 256,
                                ],
                                u8_sb[:, :, aslice],
                                rhs8,
                                start=(q == 0),
                                stop=False,
                                perf_mode=DR,
                            )
                for o in range(OB):
                    for j in range(2):
                        nc.tensor.matmul(
                            ps1[:, j * 512 : (j + 1) * 512],
                            u_sb[:, o, aslice],
                            tiles[j]["ht"][:, :, o, :],
                            start=(not f8 and o == 0),
                            stop=(o == OB - 1),
                        )

            def emit_edot(eps, tanhs, at):
                # single-instruction groups (distinct column per (at, ct)) so
                # emission can interleave with mm1 without holding PSUM
                # accumulation groups open; partials summed on DVE later.
                for ct in range(2 * TH2):
                    col = at * 2 * TH2 + ct
                    nc.tensor.matmul(
                        eps[:, col : col + 1],
                        tanhs[at][:, ct * P : (ct + 1) * P],
                        v_bf[:, at : at + 1],
                        start=True,
                        stop=True,
                    )

            def emit_mm3(b, i, tiles, pt, cps):  # noqa: cps is per-batch
                for th in range(2 * TH2):
                    j, r = th // TH2, th % TH2
                    first = i == 0 and th == 0
                    last = i == nchunk - 1 and th == 2 * TH2 - 1
                    hbf = tiles[j]["hbf"]
                    nc.tensor.matmul(
                        cps[:, 0:512],
                        pt[:, th : th + 1],
                        hbf[:, r, 0:512],
                        start=first,
                        stop=last,
                    )
                    nc.tensor.matmul(
                        cps[:, 512:1024],
                        pt[:, th : th + 1],
                        hbf[:, r, 512:1024],
                        start=first,
                        stop=last,
                    )

            def emit_finalize(b, lparts, cps):
                lsum = smpool.tile([P, 1], f32, tag="lsum")
                nc.vector.tensor_reduce(
                    out=lsum, in_=lparts, axis=Axis.X, op=Alu.add
                )
                lps = eps_pool.tile([1, 1], f32, name="lps", tag="eps")
                nc.tensor.matmul(lps, lsum, ones_sb, start=True, stop=True)
                rl = smpool.tile([1, 1], f32, tag="rl")
                nc.vector.reciprocal(rl, lps)
                o_sb = outpool.tile([1, DH], f32, tag="osb")
                nc.vector.tensor_scalar_mul(o_sb, cps, rl)
                nc.sync.dma_start(out_ext[b : b + 1, :], o_sb)

            pending = None
            pending_fin = None
            for b in range(bl):
                lparts = smpool.tile([P, nchunk], f32, tag="lparts")
                cps = cps_pool.tile([1, DH], f32, name="cps_b", tag="c")
                for i in range(nchunk):
                    tiles = (loaded.pop((b, i, 0)), loaded.pop((b, i, 1)))
                    pump(next_load + 2)
                    eps = eps_pool.tile(
                        [P, NAT * 2 * TH2], f32, name="eps", tag="eps"
                    )
                    tanhs = []
                    for at in range(NAT):
                        ps1 = mm1ps.tile([P, CHUNK_T], f32, name="ps1", tag="mm1")
                        emit_mm1(ps1, tiles, at)
                        tanh_sb = tanhpool.tile([P, CHUNK_T], bf16, tag="tanh")
                        nc.scalar.activation(
                            tanh_sb,
                            ps1,
                            Act.Tanh,
                            bias=ws_sb[:, at, b : b + 1],
                        )
                        tanhs.append(tanh_sb)
                        if at >= 1:
                            emit_edot(eps, tanhs, at - 1)
                        if at == NAT - 1:
                            if pending is not None:
                                emit_mm3(*pending)
                                if pending[1] == nchunk - 1:
                                    emit_finalize(
                                        pending[0], pending_fin, pending[4]
                                    )
                            emit_edot(eps, tanhs, at)
                    e_sb = smpool.tile([P, 2 * TH2], f32, tag="esb")
                    nc.vector.tensor_reduce(
                        out=e_sb,
                        in_=eps.rearrange("p (a c) -> p c a", a=NAT),
                        axis=Axis.X,
                        op=Alu.add,
                    )
                    pt = smpool.tile([P, 2 * TH2], bf16, tag="pt")
                    nc.scalar.activation(
                        pt, e_sb, Act.Exp, accum_out=lparts[:, i : i + 1]
                    )
                    pending = (b, i, tiles, pt, cps)
                pending_fin = lparts
            emit_mm3(*pending)
            emit_finalize(pending[0], pending_fin, pending[4])

    _legalize_waits(nc)
    return nc


def _get_nc():
    if "nc" not in _CACHE:
        _CACHE["nc"] = build_bass()
    return _CACHE["nc"]


def kernel(s, h, W_a, U_a, v_a):
    from concourse.bass_utils import run_bass_kernel_spmd

    s = np.ascontiguousarray(np.asarray(s, dtype=np.float32))
    h = np.ascontiguousarray(np.asarray(h, dtype=np.float32))
    W_a = np.ascontiguousarray(np.asarray(W_a, dtype=np.float32))
    U_a = np.ascontiguousarray(np.asarray(U_a, dtype=np.float32))
    v_a = np.ascontiguousarray(np.asarray(v_a, dtype=np.float32))

    nc = _get_nc()
    in_maps = []
    for c in range(NCORES):
        sl = slice(c * BL, (c + 1) * BL)
        in_maps.append(
            {"s": s[sl], "h": h[sl], "W_a": W_a, "U_a": U_a, "v_a": v_a}
        )
    res = run_bass_kernel_spmd(nc, in_maps, core_ids=list(range(NCORES)))
    outs = [res.results[c]["out"] for c in range(NCORES)]
    return np.concatenate(outs, axis=0).astype(np.float32)


# revision 12
# speedup vs baseline: 1.0015x; 1.0015x over previous
"""Bahdanau additive attention on 8 Trainium2 NeuronCores.

c[b] = softmax_t( tanh(s@W_a + h@U_a) @ v_a ) @ h[b]

Sharding: data-parallel over batch B=32 -> 4 batches per core; W_a, U_a,
v_a replicated. Unnormalized softmax (scores bounded by ||v_a||_1, so
f32 exp never overflows; no running max needed).

Layout: t = i*1024 + r*128 + tq (tq minor, on partitions). Per half-chunk
(512 t): 4 full-row cast DMAs f32->bf16, 4 XBAR transposes of the bf16
dh[0:768] part, and (f8 mode) a DVE cast of dh[768:1024] to fp8e4 plus a
16-bit-pair XBAR transpose whose dh-pair interleave lands exactly in the
[K,2,N] layout DoubleRow matmuls want (k = 768 + 2p + i).

Per chunk: mm1 = per (at, 256-col region): fp8 DR matmul (start) + 6 bf16
o-tile matmuls (stop); tanh on ACT with W_a@s bias; e-dot matmuls
interleaved one a-tile behind mm1 so their weight loads hide under mm1
streams; exp -> pt bf16 with accum_out partial denominators; mm3
(c += pt.T @ hbf) deferred one chunk to cover load/transpose latency.

Setup avoids SWDGE descriptor storms: s/v transposed on the PE against
tiny identity matrices; only U (and the fp8 U quarter) uses cast DMAs.
"""

import numpy as np

B, T, DH, DS, DA = 32, 4096, 1024, 1024, 512
NCORES = 8
BL = B // NCORES          # batches per core
CHUNK_T = 1024            # timesteps per compute chunk
TH2 = 4                   # 128-row groups per half-chunk
P = 128

_CACHE = {}


def _legalize_waits(nc):
    """This walrus build allows at most one sync wait per instruction.
    Tile's tail drain (and any instruction whose operands arrive via two
    DMA lanes) can carry several; split the extras onto single-wait nops
    emitted just before, in the same engine's stream."""
    from concourse import mybir

    eng_map = {}
    for eng_name in ("sync", "tensor", "vector", "scalar", "gpsimd"):
        eng = getattr(nc, eng_name)
        eng_map[eng.engine] = eng

    def make_nop(engine_type):
        bi = eng_map[engine_type].nop(nofuse=True)
        inst = bi.ins
        for fn in nc.m.functions:
            for blk in fn.blocks:
                il = list(blk.instructions)
                if il and il[-1].name == inst.name:
                    blk.instructions = il[:-1]
                    return inst
        raise RuntimeError("nop not found after emit")

    for fn in nc.m.functions:
        for blk in fn.blocks:
            insts = list(blk.instructions)
            if not any(
                getattr(i, "sync_info", None) is not None
                and len(i.sync_info.on_wait) > 1
                for i in insts
            ):
                continue
            out = []
            for inst in insts:
                si = getattr(inst, "sync_info", None)
                if si is not None and len(si.on_wait) > 1:
                    waits = list(si.on_wait)
                    for w in waits[:-1]:
                        nop = make_nop(inst.engine)
                        nop.sync_info = mybir.SyncInfo(
                            on_wait=[w], on_update=[]
                        )
                        out.append(nop)
                    inst.sync_info = mybir.SyncInfo(
                        on_wait=[waits[-1]], on_update=list(si.on_update)
                    )
                out.append(inst)
            blk.instructions = out


def build_bass(bl=BL, t_total=T, f8=True):
    import concourse.bass as bass
    import concourse.tile as tile
    from concourse import mybir

    f32 = mybir.dt.float32
    bf16 = mybir.dt.bfloat16
    fp8 = mybir.dt.float8e4
    Alu = mybir.AluOpType
    Act = mybir.ActivationFunctionType
    Axis = mybir.AxisListType
    DR = mybir.MatmulPerfMode.DoubleRow

    nchunk = t_total // CHUNK_T
    DH8 = 256 if f8 else 0    # dh columns computed in fp8 DoubleRow
    DHB = DH - DH8            # dh columns computed in bf16
    OB = DHB // P             # bf16 o-tiles (6 or 8)
    NAT = DA // P             # a-tiles (4)

    nc = bass.Bass()
    s_ext = nc.declare_dram_parameter("s", [bl, DS], f32, isOutput=False)
    h_ext = nc.declare_dram_parameter("h", [bl, t_total, DH], f32, isOutput=False)
    w_ext = nc.declare_dram_parameter("W_a", [DS, DA], f32, isOutput=False)
    u_ext = nc.declare_dram_parameter("U_a", [DH, DA], f32, isOutput=False)
    v_ext = nc.declare_dram_parameter("v_a", [DA], f32, isOutput=False)
    out_ext = nc.declare_dram_parameter("out", [bl, DH], f32, isOutput=True)

    with tile.TileContext(nc) as tc:
        from contextlib import ExitStack

        with ExitStack() as ctx:
            singles = ctx.enter_context(tc.tile_pool(name="singles", bufs=1))
            hpool = ctx.enter_context(tc.tile_pool(name="hpool", bufs=8))
            htpool = ctx.enter_context(tc.tile_pool(name="htpool", bufs=7))
            if f8:
                h8pool = ctx.enter_context(tc.tile_pool(name="h8pool", bufs=6))
                ht8pool = ctx.enter_context(
                    tc.tile_pool(name="ht8pool", bufs=7)
                )
            tanhpool = ctx.enter_context(tc.tile_pool(name="tanhpool", bufs=5))
            smpool = ctx.enter_context(tc.tile_pool(name="smpool", bufs=4))
            outpool = ctx.enter_context(tc.tile_pool(name="outpool", bufs=2))
            mm1ps = ctx.enter_context(
                tc.tile_pool(name="mm1ps", bufs=2, space="PSUM")
            )
            cps_pool = ctx.enter_context(
                tc.tile_pool(name="cps", bufs=1, space="PSUM")
            )
            eps_pool = ctx.enter_context(
                tc.tile_pool(name="epsp", bufs=2, space="PSUM")
            )

            # ---- one-time setup (all small; emitted before h loads so
            # the first chunk's compute inputs are ready ASAP) ----
            # U bf16 part: [dh_lo, o, a]
            u_sb = singles.tile([P, OB, DA], bf16)
            u_re = u_ext[:].rearrange("(o p) a -> p o a", p=P)
            for o in range(OB):
                nc.gpsimd.dma_start(u_sb[:, o, :], u_re[:, o, :])
            if f8:
                # U fp8 quarter: u8[p, i, a] = U[DHB + 2p + i, a]
                u8_sb = singles.tile([P, 2, DA], fp8)
                u8_re = u_ext[DHB:DH, :].rearrange(
                    "(p two) a -> two p a", two=2
                )
                for i in range(2):
                    nc.gpsimd.dma_start(u8_sb[:, i, :], u8_re[i])

            w_sb = singles.tile([P, DS // P, DA], bf16)
            w_re = w_ext[:].rearrange("(o p) a -> p o a", p=P)
            for o in range(DS // P):
                nc.gpsimd.dma_start(w_sb[:, o, :], w_re[:, o, :])
            s_row = singles.tile([bl, DS], f32)
            nc.sync.dma_start(s_row, s_ext[:])
            v_row = singles.tile([1, DA], f32)
            nc.sync.dma_start(v_row, v_ext[:].rearrange("(x a) -> x a", x=1))

            ones_b = singles.tile([bl, bl], f32)
            nc.any.memset(ones_b, 1.0)
            identb = singles.tile([bl, bl], f32)
            nc.gpsimd.affine_select(
                identb,
                ones_b,
                pattern=[[-1, bl]],
                compare_op=Alu.is_equal,
                fill=0.0,
                base=0,
                channel_multiplier=1,
            )
            ident1 = singles.tile([1, 1], f32)
            nc.any.memset(ident1, 1.0)
            ones_sb = singles.tile([P, 1], f32)
            nc.any.memset(ones_sb, 1.0)

            # sT via PE transpose: st[ds_lo, o, b] = s[b, o*128+ds_lo]
            ps_st = eps_pool.tile([P, DS // P, bl], f32, name="ps_st", tag="eps")
            for o in range(DS // P):
                nc.tensor.transpose(
                    ps_st[:, o, :], s_row[:, o * P : (o + 1) * P], identb
                )
            st_sb = singles.tile([P, DS // P, bl], bf16)
            nc.vector.tensor_copy(st_sb, ps_st)

            # v as [a_lo, a_hi] via PE transpose
            ps_v = eps_pool.tile([P, DA // P], f32, name="ps_v", tag="eps")
            for g in range(DA // P):
                nc.tensor.transpose(
                    ps_v[:, g : g + 1], v_row[:, g * P : (g + 1) * P], ident1
                )
            v_bf = singles.tile([P, DA // P], bf16)
            nc.vector.tensor_copy(v_bf, ps_v)

            # W_a_s^T [a_lo, at, b]
            ps_ws = eps_pool.tile([P, NAT, bl], f32, name="ps_ws", tag="eps")
            for at in range(NAT):
                for o in range(DS // P):
                    nc.tensor.matmul(
                        ps_ws[:, at, :],
                        w_sb[:, o, at * P : (at + 1) * P],
                        st_sb[:, o, :],
                        start=(o == 0),
                        stop=(o == DS // P - 1),
                    )
            ws_sb = singles.tile([P, NAT, bl], f32)
            nc.vector.tensor_copy(ws_sb, ps_ws)

            # ---- pipelined h loads (half-chunk = 512 t granularity) ----
            def emit_load(b, i, j):
                hbf = hpool.tile([P, TH2, DH], bf16, tag="hbf")
                t0 = i * CHUNK_T + j * TH2 * P
                for r in range(TH2):
                    nc.gpsimd.dma_start(
                        hbf[:, r, :],
                        h_ext[b, t0 + r * P : t0 + (r + 1) * P, :],
                    )
                # bf16 transposes, one per r (keeps the XBAR input 2D):
                # ht[dh_lo, r, o, tq] = h[t0 + r*128 + tq, o*128 + dh_lo]
                ht = htpool.tile([P, TH2, OB, P], bf16, tag="ht")
                for r in range(TH2):
                    nc.sync.dma_start_transpose(
                        ht[:, r, :, :], hbf[:, r, 0:DHB]
                    )
                if not f8:
                    return {"hbf": hbf, "ht": ht}
                # fp8 quarter: DVE cast, then 16-bit-pair XBAR transpose.
                h8 = h8pool.tile([P, TH2, DH8], fp8, tag="h8")
                nc.vector.tensor_copy(h8, hbf[:, :, DHB:DH])
                # ht8_16[p, r, tq] = pair(h8[tq, r, 2p], h8[tq, r, 2p+1])
                ht8 = ht8pool.tile([P, TH2, P], bf16, tag="ht8")
                nc.sync.dma_start_transpose(
                    ht8, h8.bitcast(bf16).rearrange("p a b -> p (a b)")
                )
                return {"hbf": hbf, "ht": ht, "ht8": ht8.bitcast(fp8)}

            halves = [
                (b, i, j)
                for b in range(bl)
                for i in range(nchunk)
                for j in range(2)
            ]
            loaded = {}
            next_load = 0

            def pump(upto):
                nonlocal next_load
                while next_load < min(upto, len(halves)):
                    key = halves[next_load]
                    loaded[key] = emit_load(*key)
                    next_load += 1

            pump(4)

            # ---- per-chunk compute ----
            def emit_mm1(ps1, tiles, at):
                # weight-reuse order: each lhsT (u8, then each o-tile) sweeps
                # all 4 output regions before the next weight load.
                aslice = slice(at * P, (at + 1) * P)
                if f8:
                    # one accumulation group per 512-col PSUM bank; the DR
                    # matmul for the first 256-half opens it (pending-zero
                    # covers the whole bank), everything else accumulates.
                    for j in range(2):
                        for q in range(2):
                            rhs8 = (
                                tiles[j]["ht8"][:, 2 * q : 2 * q + 2, :]
                                .rearrange("p r (n i) -> p i (r n)", i=2)
                            )
                            nc.tensor.matmul(
                                ps1[
                                    :,
                                    j * 512 + q * 256 : j * 512 + (q + 1) * 256,
                                ],
                                u8_sb[:, :, aslice],
                                rhs8,
                                start=(q == 0),
                                stop=False,
                                perf_mode=DR,
                            )
                for o in range(OB):
                    for j in range(2):
                        nc.tensor.matmul(
                            ps1[:, j * 512 : (j + 1) * 512],
                            u_sb[:, o, aslice],
                            tiles[j]["ht"][:, :, o, :],
                            start=(not f8 and o == 0),
                            stop=(o == OB - 1),
                        )

            def emit_edot(eps, tanhs, at):
                # single-instruction groups (distinct column per (at, ct)) so
                # emission can interleave with mm1 without holding PSUM
                # accumulation groups open; partials summed on DVE later.
                for ct in range(2 * TH2):
                    col = at * 2 * TH2 + ct
                    nc.tensor.matmul(
                        eps[:, col : col + 1],
                        tanhs[at][:, ct * P : (ct + 1) * P],
                        v_bf[:, at : at + 1],
                        start=True,
                        stop=True,
                    )

            def emit_mm3(b, i, tiles, pt, cps):  # noqa: cps is per-batch
                for th in range(2 * TH2):
                    j, r = th // TH2, th % TH2
                    first = i == 0 and th == 0
                    last = i == nchunk - 1 and th == 2 * TH2 - 1
                    hbf = tiles[j]["hbf"]
                    nc.tensor.matmul(
                        cps[:, 0:512],
                        pt[:, th : th + 1],
                        hbf[:, r, 0:512],
                        start=first,
                        stop=last,
                    )
                    nc.tensor.matmul(
                        cps[:, 512:1024],
                        pt[:, th : th + 1],
                        hbf[:, r, 512:1024],
                        start=first,
                        stop=last,
                    )

            def emit_finalize(b, lparts, cps):
                lsum = smpool.tile([P, 1], f32, tag="lsum")
                nc.vector.tensor_reduce(
                    out=lsum, in_=lparts, axis=Axis.X, op=Alu.add
                )
                lps = eps_pool.tile([1, 1], f32, name="lps", tag="eps")
                nc.tensor.matmul(lps, lsum, ones_sb, start=True, stop=True)
                rl = smpool.tile([1, 1], f32, tag="rl")
                nc.vector.reciprocal(rl, lps)
                o_sb = outpool.tile([1, DH], f32, tag="osb")
                nc.vector.tensor_scalar_mul(o_sb, cps, rl)
                nc.sync.dma_start(out_ext[b : b + 1, :], o_sb)

            pending = None
            pending_fin = None
            for b in range(bl):
                lparts = smpool.tile([P, nchunk], f32, tag="lparts")
                cps = cps_pool.tile([1, DH], f32, name="cps_b", tag="c")
                for i in range(nchunk):
                    tiles = (loaded.pop((b, i, 0)), loaded.pop((b, i, 1)))
                    pump(next_load + 2)
                    eps = eps_pool.tile(
                        [P, NAT * 2 * TH2], f32, name="eps", tag="eps"
                    )
                    tanhs = []
                    for at in range(NAT):
                        ps1 = mm1ps.tile([P, CHUNK_T], f32, name="ps1", tag="mm1")
                        emit_mm1(ps1, tiles, at)
                        tanh_sb = tanhpool.tile([P, CHUNK_T], bf16, tag="tanh")
                        nc.scalar.activation(
                            tanh_sb,
                            ps1,
                            Act.Tanh,
                            bias=ws_sb[:, at, b : b + 1],
                        )
                        tanhs.append(tanh_sb)
                        if at >= 1:
                            emit_edot(eps, tanhs, at - 1)
                        if at == NAT - 1:
                            if pending is not None:
                                emit_mm3(*pending)
                                if pending[1] == nchunk - 1:
                                    emit_finalize(
                                        pending[0], pending_fin, pending[4]
                                    )
                            emit_edot(eps, tanhs, at)
                    e_sb = smpool.tile([P, 2 * TH2], f32, tag="esb")
                    nc.vector.tensor_reduce(
                        out=e_sb,
                        in_=eps.rearrange("p (a c) -> p c a", a=NAT),
                        axis=Axis.X,
                        op=Alu.add,
                    )
                    pt = smpool.tile([P, 2 * TH2], bf16, tag="pt")
                    nc.scalar.activation(
                        pt, e_sb, Act.Exp, accum_out=lparts[:, i : i + 1]
                    )
                    pending = (b, i, tiles, pt, cps)
                pending_fin = lparts
            emit_mm3(*pending)
            emit_finalize(pending[0], pending_fin, pending[4])

    _legalize_waits(nc)
    return nc


def _get_nc():
    if "nc" not in _CACHE:
        _CACHE["nc"] = build_bass()
    return _CACHE["nc"]


def kernel(s, h, W_a, U_a, v_a):
    from concourse.bass_utils import run_bass_kernel_spmd

    s = np.ascontiguousarray(np.asarray(s, dtype=np.float32))
    h = np.ascontiguousarray(np.asarray(h, dtype=np.float32))
    W_a = np.ascontiguousarray(np.asarray(W_a, dtype=np.float32))
    U_a = np.ascontiguousarray(np.asarray(U_a, dtype=np.float32))
    v_a = np.ascontiguousarray(np.asarray(v_a, dtype=np.float32))

    nc = _get_nc()
    in_maps = []
    for c in range(NCORES):
        sl = slice(c * BL, (c + 1) * BL)
        in_maps.append(
            {"s": s[sl], "h": h[sl], "W_a": W_a, "U_a": U_a, "v_a": v_a}
        )
    res = run_bass_kernel_spmd(nc, in_maps, core_ids=list(range(NCORES)))
    outs = [res.results[c]["out"] for c in range(NCORES)]
    return np.concatenate(outs, axis=0).astype(np.float32)


# revision 13
# speedup vs baseline: 1.8119x; 1.8093x over previous
"""Bahdanau additive attention on 8 Trainium2 NeuronCores.

c[b] = softmax_t( tanh(s@W_a + h@U_a) @ v_a ) @ h[b]

Sharding: data-parallel over batch B=32 -> 4 batches per core; W_a, U_a,
v_a replicated. Everything per-core is a single pass over h[b] (64 MiB
f32) using an unnormalized softmax (scores are bounded by ||v_a||_1, so
exp() in f32 never overflows and no running max is needed).

Per-core pipeline, per (batch, t-chunk of 1024):
  1. SWDGE DMA loads h chunk HBM->SBUF casting f32->bf16 ([t_lo, th, dh]).
  2. XBAR DMA-transpose makes the dh-major copy hT [dh_lo,(th,dh_hi),t_lo].
  3. PE: scores_pre[a, t] += U_a[dh,a].T @ hT (8 dh-tiles accum in PSUM).
  4. ACT: tanh(psum + bias(W_a@s)) -> SBUF bf16 (per 128-a tile).
  5. PE: e-dot matmuls (tanh columns as lhsT, v_a as rhs) emitted one
     a-tile behind mm1 so their weight loads hide under mm1 streams;
     each writes its own eps column (single-instruction PSUM groups).
  6. DVE: e_sb[tq, ct] = sum_at eps[tq, at*8+ct].
  7. ACT: p = exp(e_sb) -> bf16, accum_out gives the softmax denominator.
  8. PE: c[1, dh] += p[t_lo,th].T @ h_bf16 (PSUM accum across chunks,
     deferred one chunk to cover load/transpose latency).
  9. Finalize per batch: l = sum(p) via ones-matmul, c * (1/l) -> out.

vs the 460 us baseline: the one-time setup (U/W/s/v loads, W_a@s) is
emitted BEFORE the h preloads and the s/v transposes run on the PE
against tiny identity matrices instead of 4-byte-descriptor SWDGE
storms, removing most of the ~75 us startup serialization; e-dot weight
loads are interleaved under mm1 streams; three chunks are preloaded.
"""

import numpy as np

B, T, DH, DS, DA = 32, 4096, 1024, 1024, 512
NCORES = 8
BL = B // NCORES          # batches per core
CHUNK_T = 1024            # timesteps per pipeline chunk
TH = CHUNK_T // 128       # 128-row subtiles per chunk
P = 128

_CACHE = {}


def _legalize_waits(nc):
    """This walrus build allows at most one sync wait per instruction.
    Tile's tail drain (and any instruction whose operands arrive via two
    DMA lanes) can carry several; split the extras onto single-wait nops
    emitted just before, in the same engine's stream."""
    from concourse import mybir

    eng_map = {}
    for eng_name in ("sync", "tensor", "vector", "scalar", "gpsimd"):
        eng = getattr(nc, eng_name)
        eng_map[eng.engine] = eng

    def make_nop(engine_type):
        bi = eng_map[engine_type].nop(nofuse=True)
        inst = bi.ins
        for fn in nc.m.functions:
            for blk in fn.blocks:
                il = list(blk.instructions)
                if il and il[-1].name == inst.name:
                    blk.instructions = il[:-1]
                    return inst
        raise RuntimeError("nop not found after emit")

    for fn in nc.m.functions:
        for blk in fn.blocks:
            insts = list(blk.instructions)
            if not any(
                getattr(i, "sync_info", None) is not None
                and len(i.sync_info.on_wait) > 1
                for i in insts
            ):
                continue
            out = []
            for inst in insts:
                si = getattr(inst, "sync_info", None)
                if si is not None and len(si.on_wait) > 1:
                    waits = list(si.on_wait)
                    for w in waits[:-1]:
                        nop = make_nop(inst.engine)
                        nop.sync_info = mybir.SyncInfo(
                            on_wait=[w], on_update=[]
                        )
                        out.append(nop)
                    inst.sync_info = mybir.SyncInfo(
                        on_wait=[waits[-1]], on_update=list(si.on_update)
                    )
                out.append(inst)
            blk.instructions = out


def build_bass(bl=BL, t_total=T, stage=4):
    import concourse.bass as bass
    import concourse.tile as tile
    from concourse import mybir

    f32 = mybir.dt.float32
    bf16 = mybir.dt.bfloat16
    Alu = mybir.AluOpType
    Act = mybir.ActivationFunctionType
    Axis = mybir.AxisListType

    nchunk = t_total // CHUNK_T
    NAT = DA // P

    nc = bass.Bass()
    s_ext = nc.declare_dram_parameter("s", [bl, DS], f32, isOutput=False)
    h_ext = nc.declare_dram_parameter("h", [bl, t_total, DH], f32, isOutput=False)
    w_ext = nc.declare_dram_parameter("W_a", [DS, DA], f32, isOutput=False)
    u_ext = nc.declare_dram_parameter("U_a", [DH, DA], f32, isOutput=False)
    v_ext = nc.declare_dram_parameter("v_a", [DA], f32, isOutput=False)
    out_ext = nc.declare_dram_parameter("out", [bl, DH], f32, isOutput=True)

    with tile.TileContext(nc) as tc:
        from contextlib import ExitStack

        with ExitStack() as ctx:
            singles = ctx.enter_context(tc.tile_pool(name="singles", bufs=1))
            hpool = ctx.enter_context(tc.tile_pool(name="hpool", bufs=4))
            htpool = ctx.enter_context(tc.tile_pool(name="htpool", bufs=3))
            tanhpool = ctx.enter_context(tc.tile_pool(name="tanhpool", bufs=5))
            smpool = ctx.enter_context(tc.tile_pool(name="smpool", bufs=3))
            outpool = ctx.enter_context(tc.tile_pool(name="outpool", bufs=2))
            mm1ps = ctx.enter_context(
                tc.tile_pool(name="mm1ps", bufs=2, space="PSUM")
            )
            cps_pool = ctx.enter_context(
                tc.tile_pool(name="cps", bufs=1, space="PSUM")
            )
            eps_pool = ctx.enter_context(
                tc.tile_pool(name="epsp", bufs=2, space="PSUM")
            )
            tinyps = eps_pool

            # ---- one-time setup, emitted FIRST so the gpsimd/sync queues
            # deliver the mm1 weights before the bulky h chunks ----
            # U_a as bf16 [dh_lo, dh_hi, a] (lhsT tiles for the score matmul)
            # NB: SWDGE cast DMAs deadlock with 3D APs on this runtime —
            # keep every casting DMA 2D.
            u_sb = singles.tile([P, DH // P, DA], bf16)
            u_re = u_ext[:].rearrange("(o p) a -> p o a", p=P)
            for o in range(DH // P):
                nc.gpsimd.dma_start(u_sb[:, o, :], u_re[:, o, :])
            # W_a f32 [ds_lo, ds_hi, a] (lhsT tiles for the W_a@s matmul)
            w_sb = singles.tile([P, DS // P, DA], f32)
            nc.sync.dma_start(w_sb, w_ext[:].rearrange("(o p) a -> p o a", p=P))
            # s [bl, DS] f32
            s_sb = singles.tile([bl, DS], f32)
            nc.sync.dma_start(s_sb, s_ext[:])
            # v_a as a contiguous row (transposed on the PE below)
            v_row = singles.tile([1, DA], f32)
            nc.sync.dma_start(v_row, v_ext[:].rearrange("(x a) -> x a", x=1))

            ones_b = singles.tile([bl, bl], f32)
            nc.any.memset(ones_b, 1.0)
            identb = singles.tile([bl, bl], f32)
            nc.gpsimd.affine_select(
                identb,
                ones_b,
                pattern=[[-1, bl]],
                compare_op=Alu.is_equal,
                fill=0.0,
                base=0,
                channel_multiplier=1,
            )
            ident1 = singles.tile([1, 1], f32)
            nc.any.memset(ident1, 1.0)
            ones_sb = singles.tile([P, 1], f32)
            nc.any.memset(ones_sb, 1.0)

            # sT [ds_lo, ds_hi, b] via PE transpose (no descriptor storms)
            ps_st = tinyps.tile([P, DS // P, bl], f32, name="ps_st", tag="eps")
            for o in range(DS // P):
                nc.tensor.transpose(
                    ps_st[:, o, :], s_sb[:, o * P : (o + 1) * P], identb
                )
            st_sb = singles.tile([P, DS // P, bl], f32)
            nc.vector.tensor_copy(st_sb, ps_st)

            # v as [a_lo, a_hi] bf16 via PE transpose
            ps_v = tinyps.tile([P, DA // P], f32, name="ps_v", tag="eps")
            for g in range(DA // P):
                nc.tensor.transpose(
                    ps_v[:, g : g + 1], v_row[:, g * P : (g + 1) * P], ident1
                )
            v_bf = singles.tile([P, DA // P], bf16)
            nc.vector.tensor_copy(v_bf, ps_v)

            # W_a_s^T [a_lo, a_hi, b] = sum_ds W_a[ds, a] * s[b, ds]
            ps_ws = tinyps.tile([P, DA // P, bl], f32, name="ps_ws", tag="eps")
            for at in range(DA // P):
                for o in range(DS // P):
                    nc.tensor.matmul(
                        ps_ws[:, at, :],
                        w_sb[:, o, at * P : (at + 1) * P],
                        st_sb[:, o, :],
                        start=(o == 0),
                        stop=(o == DS // P - 1),
                    )
            ws_sb = singles.tile([P, DA // P, bl], f32)
            nc.vector.tensor_copy(ws_sb, ps_ws)

            # ---- h chunk pipeline ----
            def emit_load(b, i):
                # t within the chunk decomposes as t = tq*8 + tr, so the
                # cast DMA is 2D: partition tq strides 8 DRAM rows, and
                # (tr, d) is one contiguous 32 KB run.
                # hbf[tq, tr, d] = h[b, i*CHUNK_T + tq*8 + tr, d]
                hbf = hpool.tile([P, TH, DH], bf16, tag="hbf")
                nc.gpsimd.dma_start(
                    hbf.rearrange("p a b -> p (a b)"),
                    h_ext[
                        b, i * CHUNK_T : (i + 1) * CHUNK_T, :
                    ].rearrange("(tq tr) d -> tq (tr d)", tr=TH),
                )
                # xbar: ht[p, q, f] = hbf_2d[f, q*128+p]
                # => ht[dh_lo, (tr, o), tq] = h[.., tq*8 + tr, o*128 + dh_lo]
                ht = htpool.tile([P, TH, DH // P, P], bf16, tag="ht")
                nc.sync.dma_start_transpose(ht, hbf)
                return hbf, ht

            chunks = [(b, i) for b in range(bl) for i in range(nchunk)]
            loaded = {}
            next_load = 0

            def pump(upto):
                nonlocal next_load
                while next_load < min(upto, len(chunks)):
                    key = chunks[next_load]
                    loaded[key] = emit_load(*key)
                    next_load += 1

            pump(3)

            # ---- per-chunk compute ----
            def emit_mm1(ps1, ht, at):
                aslice = slice(at * P, (at + 1) * P)
                for o in range(DH // P):
                    lhsT = u_sb[:, o, aslice]
                    nc.tensor.matmul(
                        ps1[:, 0:512],
                        lhsT,
                        ht[:, 0 : TH // 2, o, :],
                        start=(o == 0),
                        stop=(o == DH // P - 1),
                    )
                    nc.tensor.matmul(
                        ps1[:, 512:1024],
                        lhsT,
                        ht[:, TH // 2 : TH, o, :],
                        start=(o == 0),
                        stop=(o == DH // P - 1),
                    )

            def emit_edot(eps, tanhs, at):
                # single-instruction groups (distinct column per (at, ct));
                # partials summed on the DVE before exp. Emitted one a-tile
                # behind mm1 so weight loads hide under mm1 streams.
                for ct in range(TH):
                    col = at * TH + ct
                    nc.tensor.matmul(
                        eps[:, col : col + 1],
                        tanhs[at][:, ct * P : (ct + 1) * P],
                        v_bf[:, at : at + 1],
                        start=True,
                        stop=True,
                    )

            def emit_mm3(b, i, hbf, pt, cps):
                for th in range(TH):
                    first = i == 0 and th == 0
                    last = i == nchunk - 1 and th == TH - 1
                    nc.tensor.matmul(
                        cps[:, 0:512],
                        pt[:, th : th + 1],
                        hbf[:, th, 0:512],
                        start=first,
                        stop=last,
                    )
                    nc.tensor.matmul(
                        cps[:, 512:1024],
                        pt[:, th : th + 1],
                        hbf[:, th, 512:1024],
                        start=first,
                        stop=last,
                    )

            def emit_finalize(b, lparts, cps):
                lsum = smpool.tile([P, 1], f32, tag="lsum")
                nc.vector.tensor_reduce(
                    out=lsum, in_=lparts, axis=Axis.X, op=Alu.add
                )
                lps = tinyps.tile([1, 1], f32, name="lps", tag="eps")
                nc.tensor.matmul(lps, lsum, ones_sb, start=True, stop=True)
                rl = smpool.tile([1, 1], f32, tag="rl")
                nc.vector.reciprocal(rl, lps)
                o_sb = outpool.tile([1, DH], f32, tag="osb")
                nc.vector.tensor_scalar_mul(o_sb, cps, rl)
                nc.sync.dma_start(out_ext[b : b + 1, :], o_sb)

            pending = None
            pending_fin = None
            for b in range(bl):
                lparts = smpool.tile([P, nchunk], f32, tag="lparts")
                cps = cps_pool.tile([1, DH], f32, name="cps_b", tag="c")
                for i in range(nchunk):
                    hbf, ht = loaded.pop((b, i))
                    pump(next_load + 1)
                    eps = eps_pool.tile(
                        [P, NAT * TH], f32, name="eps", tag="eps"
                    )
                    tanhs = []
                    for at in range(NAT):
                        ps1 = mm1ps.tile(
                            [P, CHUNK_T], f32, name="ps1", tag="mm1"
                        )
                        emit_mm1(ps1, ht, at)
                        tanh_sb = tanhpool.tile([P, CHUNK_T], bf16, tag="tanh")
                        nc.scalar.activation(
                            tanh_sb,
                            ps1,
                            Act.Tanh,
                            bias=ws_sb[:, at, b : b + 1],
                        )
                        tanhs.append(tanh_sb)
                        if at >= 1:
                            emit_edot(eps, tanhs, at - 1)
                        if at == NAT - 1:
                            if pending is not None:
                                emit_mm3(*pending)
                                if pending[1] == nchunk - 1:
                                    emit_finalize(
                                        pending[0], pending_fin, pending[4]
                                    )
                            emit_edot(eps, tanhs, at)
                    e_sb = smpool.tile([P, TH], f32, tag="esb")
                    nc.vector.tensor_reduce(
                        out=e_sb,
                        in_=eps.rearrange("p (a c) -> p c a", a=NAT),
                        axis=Axis.X,
                        op=Alu.add,
                    )
                    pt = smpool.tile([P, TH], bf16, tag="pt")
                    nc.scalar.activation(
                        pt, e_sb, Act.Exp, accum_out=lparts[:, i : i + 1]
                    )
                    pending = (b, i, hbf, pt, cps)
                pending_fin = lparts
            emit_mm3(*pending)
            emit_finalize(pending[0], pending_fin, pending[4])

    _legalize_waits(nc)
    return nc


def _get_nc():
    if "nc" not in _CACHE:
        _CACHE["nc"] = build_bass()
    return _CACHE["nc"]


def kernel(s, h, W_a, U_a, v_a):
    from concourse.bass_utils import run_bass_kernel_spmd

    s = np.ascontiguousarray(np.asarray(s, dtype=np.float32))
    h = np.ascontiguousarray(np.asarray(h, dtype=np.float32))
    W_a = np.ascontiguousarray(np.asarray(W_a, dtype=np.float32))
    U_a = np.ascontiguousarray(np.asarray(U_a, dtype=np.float32))
    v_a = np.ascontiguousarray(np.asarray(v_a, dtype=np.float32))

    nc = _get_nc()
    in_maps = []
    for c in range(NCORES):
        sl = slice(c * BL, (c + 1) * BL)
        in_maps.append(
            {"s": s[sl], "h": h[sl], "W_a": W_a, "U_a": U_a, "v_a": v_a}
        )
    res = run_bass_kernel_spmd(nc, in_maps, core_ids=list(range(NCORES)))
    outs = [res.results[c]["out"] for c in range(NCORES)]
    return np.concatenate(outs, axis=0).astype(np.float32)
